# revision 9
# baseline (speedup 1.0000x reference)
"""BPLoss Trainium2 kernel: 8-core SPMD over the detection (N) axis.

v5 design. Per core (shard of R=12544 rows; partition p owns rows
p*98..p*98+97, so group DMAs read contiguous multi-KiB runs per
partition):
  - tile groups (thirteen 7-tile groups + 3/2/2 tail) are DMA'd as two
    plain HWDGE transfers each -- a DVE part [128,d,1024] on one ring
    and a Pool part [128,p,1024] on the other (dual rings measured at
    ~400 GB/s aggregate)
  - masking per 128-row tile (HW-microbenched):
      DVE part:  masked = (iota != label) * cs  (fused stt, 1.28 us)
                 -> scratch, then per-part reduce_max
      Pool part: ONE batched in-place GpSimd multiply by a bf16 {0,1}
                 mask block nv[128,p,1024] (2.12 us/tile), then one
                 contiguous grouped reduce_max
    nv masks depend only on the label table, so they are built ahead of
    the data: ACT (Square -> Sign, 2 passes) for most tiles, DVE
    tensor_scalar (0.74 us, 2x mode) for one tile per group to keep ACT
    under the DMA pace
  - epilogue: Ln on ScalarE, fused multiply-accumulate dot products for
    sum((z+r)*log_max) and sum(z*||xywh - gt_xywh[idx]||^2)
Host: gathers the tiny gt tables per row (labels, gt_xywh[idx]), shards,
pads core 7, sums the 8x[128,2] partials, combines -A + exp(-B).
"""
import numpy as np
import concourse.bass as bass
import concourse.tile as tile
from concourse import bacc, mybir
from concourse.bass_utils import run_bass_kernel_spmd

N, C, M = 100000, 1024, 128
NCORES = 8
T = 98              # 128-row tiles per core
R = T * 128         # 12544 rows per core
# (group_size, n_dve_tiles): 13x7 steady groups + short tail groups
GROUPS = [(7, 2)] * 13 + [(3, 1), (2, 1), (2, 1)]
assert sum(g for g, _ in GROUPS) == T
CS_BUFS = 3
NV_BUFS = 3
U_BUFS = 4
MSK_BUFS = 4

f32 = mybir.dt.float32
bf16 = mybir.dt.bfloat16
OP = mybir.AluOpType
AF = mybir.ActivationFunctionType
AX = mybir.AxisListType

# packed f32 per-row tables: [label | -label | z | r | xywh | g | iota]
PF_LAB = 0
PF_NLAB = T
PF_Z = 2 * T
PF_R = 3 * T
PF_XYWH = 4 * T
PF_G = 8 * T
PF_IOTA = 12 * T
PF_COLS = 12 * T + C


def build_nc(reps=1):
    nc = bacc.Bacc("TRN2", target_bir_lowering=False, debug=False,
                   num_devices=NCORES)
    cs_d = nc.dram_tensor("cs", [128, T * C], f32, kind="ExternalInput").ap()
    pf_d = nc.dram_tensor("pf", [128, PF_COLS], f32, kind="ExternalInput").ap()
    out_d = nc.dram_tensor("out", [128, 2], f32, kind="ExternalOutput").ap()

    with tile.TileContext(nc) as tc:
        with (
            tc.tile_pool(name="const", bufs=1) as constp,
            tc.tile_pool(name="dvep", bufs=CS_BUFS) as dvep,
            tc.tile_pool(name="poolp", bufs=CS_BUFS) as poolp,
            tc.tile_pool(name="nvp", bufs=NV_BUFS) as nvp,
            tc.tile_pool(name="up", bufs=U_BUFS) as up,
            tc.tile_pool(name="mskp", bufs=MSK_BUFS) as mskp,
        ):
            pf = constp.tile([128, PF_COLS], f32)
            nc.scalar.dma_start(out=pf[:], in_=pf_d[:])
            lab = pf[:, PF_LAB : PF_LAB + T]
            nlab = pf[:, PF_NLAB : PF_NLAB + T]
            z_sb = pf[:, PF_Z : PF_Z + T]
            r_sb = pf[:, PF_R : PF_R + T]
            xywh_sb = pf[:, PF_XYWH : PF_XYWH + 4 * T].rearrange(
                "p (t c) -> p t c", c=4
            )
            g_sb = pf[:, PF_G : PF_G + 4 * T].rearrange("p (t c) -> p t c", c=4)
            iota = pf[:, PF_IOTA : PF_IOTA + C]

            w_sb = constp.tile([128, T], f32)
            nc.vector.tensor_add(w_sb[:], z_sb, r_sb)
            rowmax = constp.tile([128, T], f32)
            lm = constp.tile([128, T], f32)
            out_sb = constp.tile([128, 2], f32)
            scr = constp.tile([128, T], f32)
            scr2 = constp.tile([128, T], f32)
            diff = constp.tile([128, T, 4], f32)
            dsum = constp.tile([128, T], f32)

            for rep in range(reps):
                for gi, (gsz, nd) in enumerate(GROUPS):
                    t0 = sum(g for g, _ in GROUPS[:gi])
                    npl = gsz - nd          # pool-class tiles in this group
                    td = t0 + nd            # first pool-class tile

                    # bf16 {0,1} mask block for the pool part (pf-only dep;
                    # built ahead of the data DMA). One mask per group is
                    # built on DVE (2x tensor_scalar), the rest on ACT.
                    nv = nvp.tile([128, npl, C], bf16)
                    for k in range(npl):
                        t = td + k
                        if k == 0 and gi % 2 == 0:
                            nc.vector.tensor_scalar(
                                out=nv[:, k, :], in0=iota,
                                scalar1=lab[:, t : t + 1], scalar2=None,
                                op0=OP.not_equal,
                            )
                        else:
                            u = up.tile([128, C], f32)
                            nc.scalar.activation(
                                out=u[:], in_=iota, func=AF.Square,
                                scale=1.0, bias=nlab[:, t : t + 1],
                            )
                            nc.scalar.activation(
                                out=nv[:, k, :], in_=u[:], func=AF.Sign
                            )

                    dvw = dvep.tile([128, nd, C], f32)
                    nc.sync.dma_start(
                        out=dvw[:],
                        in_=cs_d[:, t0 * C : td * C].rearrange(
                            "p (a c) -> p a c", c=C
                        ),
                    )
                    plw = poolp.tile([128, npl, C], f32)
                    nc.scalar.dma_start(
                        out=plw[:],
                        in_=cs_d[:, td * C : (td + npl) * C].rearrange(
                            "p (a c) -> p a c", c=C
                        ),
                    )

                    msk = mskp.tile([128, nd, C], f32)
                    for h in range(nd):
                        t = t0 + h
                        nc.vector.scalar_tensor_tensor(
                            out=msk[:, h, :], in0=iota,
                            scalar=lab[:, t : t + 1],
                            in1=dvw[:, h, :], op0=OP.not_equal, op1=OP.mult,
                        )
                    nc.vector.reduce_max(
                        rowmax[:, t0 : t0 + nd], msk[:], axis=AX.X
                    )
                    nc.gpsimd.tensor_tensor(
                        out=plw[:], in0=plw[:], in1=nv[:], op=OP.mult,
                    )
                    nc.vector.reduce_max(
                        rowmax[:, td : td + npl], plw[:], axis=AX.X
                    )

                # epilogue: partial sums
                nc.scalar.activation(out=lm[:], in_=rowmax[:], func=AF.Ln)
                nc.vector.scalar_tensor_tensor(
                    out=scr[:], in0=w_sb[:], scalar=0.0, in1=lm[:],
                    op0=OP.bypass, op1=OP.mult, accum_out=out_sb[:, 0:1],
                )
                nc.vector.tensor_sub(diff[:], xywh_sb, g_sb)
                nc.vector.tensor_mul(diff[:], diff[:], diff[:])
                nc.vector.reduce_sum(dsum[:], diff[:], axis=AX.X)
                nc.vector.scalar_tensor_tensor(
                    out=scr2[:], in0=z_sb, scalar=0.0, in1=dsum[:],
                    op0=OP.bypass, op1=OP.mult, accum_out=out_sb[:, 1:2],
                )
            nc.sync.dma_start(out=out_d[:], in_=out_sb[:])

    nc.compile()
    return nc


def make_in_maps(class_scores, xywh, z, r, nearest_gt_idx, gt_class_labels, gt_xywh):
    cs = np.ascontiguousarray(np.asarray(class_scores, dtype=np.float32))
    xywh = np.ascontiguousarray(np.asarray(xywh, dtype=np.float32))
    z = np.ascontiguousarray(np.asarray(z, dtype=np.float32))
    r = np.ascontiguousarray(np.asarray(r, dtype=np.float32))
    idx = np.asarray(nearest_gt_idx).astype(np.int64)
    labels = np.asarray(gt_class_labels).astype(np.float32)[idx]       # [N]
    gx = np.asarray(gt_xywh, dtype=np.float32)[idx]                    # [N,4]

    iota_row = np.arange(C, dtype=np.float32)[None, :]
    in_maps = []
    for c in range(NCORES):
        lo, hi = c * R, (c + 1) * R
        if hi <= N:
            cs_s = cs[lo:hi]
            lab_s, z_s, r_s = labels[lo:hi], z[lo:hi], r[lo:hi]
            xywh_s, gx_s = xywh[lo:hi], gx[lo:hi]
        else:
            n_real = N - lo
            cs_s = np.ones((R, C), dtype=np.float32)
            cs_s[:n_real] = cs[lo:]
            lab_s = np.zeros(R, np.float32); lab_s[:n_real] = labels[lo:]
            z_s = np.zeros(R, np.float32); z_s[:n_real] = z[lo:]
            r_s = np.zeros(R, np.float32); r_s[:n_real] = r[lo:]
            xywh_s = np.zeros((R, 4), np.float32); xywh_s[:n_real] = xywh[lo:]
            gx_s = np.zeros((R, 4), np.float32); gx_s[:n_real] = gx[lo:]
        pf = np.empty((128, PF_COLS), dtype=np.float32)
        pf[:, PF_LAB : PF_LAB + T] = lab_s.reshape(128, T)
        pf[:, PF_NLAB : PF_NLAB + T] = -lab_s.reshape(128, T)
        pf[:, PF_Z : PF_Z + T] = z_s.reshape(128, T)
        pf[:, PF_R : PF_R + T] = r_s.reshape(128, T)
        pf[:, PF_XYWH : PF_XYWH + 4 * T] = xywh_s.reshape(128, 4 * T)
        pf[:, PF_G : PF_G + 4 * T] = gx_s.reshape(128, 4 * T)
        pf[:, PF_IOTA : PF_IOTA + C] = iota_row
        in_maps.append({"cs": cs_s.reshape(128, T * C), "pf": pf})
    return in_maps


def combine_outputs(outs):
    """outs: list of [128, 2] per-core partials -> final [1] float32."""
    partA = float(sum(o[:, 0].astype(np.float64).sum() for o in outs))
    partB = float(sum(o[:, 1].astype(np.float64).sum() for o in outs))
    with np.errstate(over="ignore", under="ignore"):
        tps = np.exp(-partB)
    val = -partA + tps
    return np.array([val], dtype=np.float32)


_NC_CACHE = None


def get_nc():
    global _NC_CACHE
    if _NC_CACHE is None:
        _NC_CACHE = build_nc()
    return _NC_CACHE


def kernel(**inputs) -> np.ndarray:
    nc = get_nc()
    in_maps = make_in_maps(**inputs)
    res = run_bass_kernel_spmd(nc, in_maps, core_ids=list(range(NCORES)))
    return combine_outputs([res.results[c]["out"] for c in range(NCORES)])


# revision 12
# speedup vs baseline: 1.1992x; 1.1992x over previous
"""BPLoss Trainium2 kernel: 8-core SPMD over the detection (N) axis.

v5 design. Per core (shard of R=12544 rows; partition p owns rows
p*98..p*98+97, so group DMAs read contiguous multi-KiB runs per
partition):
  - tile groups (thirteen 7-tile groups + 3/2/2 tail) are DMA'd as two
    plain HWDGE transfers each -- a DVE part [128,d,1024] on one ring
    and a Pool part [128,p,1024] on the other (dual rings measured at
    ~400 GB/s aggregate)
  - masking per 128-row tile (HW-microbenched):
      DVE part:  masked = (iota != label) * cs  (fused stt, 1.28 us)
                 -> scratch, then per-part reduce_max
      Pool part: ONE batched in-place GpSimd multiply by a bf16 {0,1}
                 mask block nv[128,p,1024] (2.12 us/tile), then one
                 contiguous grouped reduce_max
    nv masks depend only on the label table, so they are built ahead of
    the data: ACT (Square -> Sign, 2 passes) for most tiles, DVE
    tensor_scalar (0.74 us, 2x mode) for one tile per group to keep ACT
    under the DMA pace
  - epilogue: Ln on ScalarE, fused multiply-accumulate dot products for
    sum((z+r)*log_max) and sum(z*||xywh - gt_xywh[idx]||^2)
Host: gathers the tiny gt tables per row (labels, gt_xywh[idx]), shards,
pads core 7, sums the 8x[128,2] partials, combines -A + exp(-B).
"""
import numpy as np
import concourse.bass as bass
import concourse.tile as tile
from concourse import bacc, mybir
from concourse.bass_utils import run_bass_kernel_spmd

N, C, M = 100000, 1024, 128
NCORES = 8
T = 98              # 128-row tiles per core
R = T * 128         # 12544 rows per core
# (group_size, n_dve_tiles): 13x7 steady groups + short tail groups
GROUPS = [(7, 2)] * 13 + [(3, 1), (2, 1), (2, 1)]
assert sum(g for g, _ in GROUPS) == T
POOL_BUFS = 4       # deferred grouped reduce keeps one extra alive
DVE_BUFS = 3
NV_BUFS = 3
U_BUFS = 4
MSK_BUFS = 3

f32 = mybir.dt.float32
bf16 = mybir.dt.bfloat16
OP = mybir.AluOpType
AF = mybir.ActivationFunctionType
AX = mybir.AxisListType

# packed f32 per-row tables: [label | -label | z | r | xywh | g | iota]
PF_LAB = 0
PF_NLAB = T
PF_Z = 2 * T
PF_R = 3 * T
PF_XYWH = 4 * T
PF_G = 8 * T
PF_IOTA = 12 * T
PF_COLS = 12 * T + C


def build_nc(reps=1):
    nc = bacc.Bacc("TRN2", target_bir_lowering=False, debug=False,
                   num_devices=NCORES)
    cs_d = nc.dram_tensor("cs", [128, T * C], f32, kind="ExternalInput").ap()
    pf_d = nc.dram_tensor("pf", [128, PF_COLS], f32, kind="ExternalInput").ap()
    out_d = nc.dram_tensor("out", [128, 2], f32, kind="ExternalOutput").ap()

    with tile.TileContext(nc) as tc:
        with (
            tc.tile_pool(name="const", bufs=1) as constp,
            tc.tile_pool(name="dvep", bufs=DVE_BUFS) as dvep,
            tc.tile_pool(name="poolp", bufs=POOL_BUFS) as poolp,
            tc.tile_pool(name="nvp", bufs=NV_BUFS) as nvp,
            tc.tile_pool(name="up", bufs=U_BUFS) as up,
            tc.tile_pool(name="mskp", bufs=MSK_BUFS) as mskp,
        ):
            pf = constp.tile([128, PF_COLS], f32)
            nc.scalar.dma_start(out=pf[:], in_=pf_d[:])
            lab = pf[:, PF_LAB : PF_LAB + T]
            nlab = pf[:, PF_NLAB : PF_NLAB + T]
            z_sb = pf[:, PF_Z : PF_Z + T]
            r_sb = pf[:, PF_R : PF_R + T]
            xywh_sb = pf[:, PF_XYWH : PF_XYWH + 4 * T].rearrange(
                "p (t c) -> p t c", c=4
            )
            g_sb = pf[:, PF_G : PF_G + 4 * T].rearrange("p (t c) -> p t c", c=4)
            iota = pf[:, PF_IOTA : PF_IOTA + C]

            w_sb = constp.tile([128, T], f32)
            nc.vector.tensor_add(w_sb[:], z_sb, r_sb)
            rowmax = constp.tile([128, T], f32)
            lm = constp.tile([128, T], f32)
            out_sb = constp.tile([128, 2], f32)
            scr = constp.tile([128, T], f32)
            scr2 = constp.tile([128, T], f32)
            diff = constp.tile([128, T, 4], f32)
            dsum = constp.tile([128, T], f32)

            for rep in range(reps):
                pend = None   # deferred grouped reduce (plw, td, npl)
                for gi, (gsz, nd) in enumerate(GROUPS):
                    t0 = sum(g for g, _ in GROUPS[:gi])
                    npl = gsz - nd          # pool-class tiles in this group
                    td = t0 + nd            # first pool-class tile

                    # bf16 {0,1} mask block for the pool part (pf-only dep;
                    # built ahead of the data DMA). One mask per group is
                    # built on DVE (2x tensor_scalar), the rest on ACT.
                    nv = nvp.tile([128, npl, C], bf16)
                    for k in range(npl):
                        t = td + k
                        if k == 0 and gi % 2 == 0:
                            nc.vector.tensor_scalar(
                                out=nv[:, k, :], in0=iota,
                                scalar1=lab[:, t : t + 1], scalar2=None,
                                op0=OP.not_equal,
                            )
                        else:
                            u = up.tile([128, C], f32)
                            nc.scalar.activation(
                                out=u[:], in_=iota, func=AF.Square,
                                scale=1.0, bias=nlab[:, t : t + 1],
                            )
                            nc.scalar.activation(
                                out=nv[:, k, :], in_=u[:], func=AF.Sign
                            )

                    # both data DMAs on the sync HWDGE ring: the pool part
                    # first (it heads the longer chain)
                    plw = poolp.tile([128, npl, C], f32)
                    nc.sync.dma_start(
                        out=plw[:],
                        in_=cs_d[:, td * C : (td + npl) * C].rearrange(
                            "p (a c) -> p a c", c=C
                        ),
                    )
                    dvw = dvep.tile([128, nd, C], f32)
                    nc.sync.dma_start(
                        out=dvw[:],
                        in_=cs_d[:, t0 * C : td * C].rearrange(
                            "p (a c) -> p a c", c=C
                        ),
                    )

                    msk = mskp.tile([128, nd, C], f32)
                    for h in range(nd):
                        t = t0 + h
                        nc.vector.scalar_tensor_tensor(
                            out=msk[:, h, :], in0=iota,
                            scalar=lab[:, t : t + 1],
                            in1=dvw[:, h, :], op0=OP.not_equal, op1=OP.mult,
                        )
                    nc.vector.reduce_max(
                        rowmax[:, t0 : t0 + nd], msk[:], axis=AX.X
                    )
                    nc.gpsimd.tensor_tensor(
                        out=plw[:], in0=plw[:], in1=nv[:], op=OP.mult,
                    )
                    # grouped reduce is deferred one group so the DVE's
                    # in-order queue never stalls waiting on GpSimd
                    if pend is not None:
                        pplw, ptd, pnpl = pend
                        nc.vector.reduce_max(
                            rowmax[:, ptd : ptd + pnpl], pplw[:], axis=AX.X
                        )
                    pend = (plw, td, npl)
                pplw, ptd, pnpl = pend
                nc.vector.reduce_max(
                    rowmax[:, ptd : ptd + pnpl], pplw[:], axis=AX.X
                )

                # epilogue: partial sums
                nc.scalar.activation(out=lm[:], in_=rowmax[:], func=AF.Ln)
                nc.vector.scalar_tensor_tensor(
                    out=scr[:], in0=w_sb[:], scalar=0.0, in1=lm[:],
                    op0=OP.bypass, op1=OP.mult, accum_out=out_sb[:, 0:1],
                )
                nc.vector.tensor_sub(diff[:], xywh_sb, g_sb)
                nc.vector.tensor_mul(diff[:], diff[:], diff[:])
                nc.vector.reduce_sum(dsum[:], diff[:], axis=AX.X)
                nc.vector.scalar_tensor_tensor(
                    out=scr2[:], in0=z_sb, scalar=0.0, in1=dsum[:],
                    op0=OP.bypass, op1=OP.mult, accum_out=out_sb[:, 1:2],
                )
            nc.sync.dma_start(out=out_d[:], in_=out_sb[:])

    nc.compile()
    return nc


def make_in_maps(class_scores, xywh, z, r, nearest_gt_idx, gt_class_labels, gt_xywh):
    cs = np.ascontiguousarray(np.asarray(class_scores, dtype=np.float32))
    xywh = np.ascontiguousarray(np.asarray(xywh, dtype=np.float32))
    z = np.ascontiguousarray(np.asarray(z, dtype=np.float32))
    r = np.ascontiguousarray(np.asarray(r, dtype=np.float32))
    idx = np.asarray(nearest_gt_idx).astype(np.int64)
    labels = np.asarray(gt_class_labels).astype(np.float32)[idx]       # [N]
    gx = np.asarray(gt_xywh, dtype=np.float32)[idx]                    # [N,4]

    iota_row = np.arange(C, dtype=np.float32)[None, :]
    in_maps = []
    for c in range(NCORES):
        lo, hi = c * R, (c + 1) * R
        if hi <= N:
            cs_s = cs[lo:hi]
            lab_s, z_s, r_s = labels[lo:hi], z[lo:hi], r[lo:hi]
            xywh_s, gx_s = xywh[lo:hi], gx[lo:hi]
        else:
            n_real = N - lo
            cs_s = np.ones((R, C), dtype=np.float32)
            cs_s[:n_real] = cs[lo:]
            lab_s = np.zeros(R, np.float32); lab_s[:n_real] = labels[lo:]
            z_s = np.zeros(R, np.float32); z_s[:n_real] = z[lo:]
            r_s = np.zeros(R, np.float32); r_s[:n_real] = r[lo:]
            xywh_s = np.zeros((R, 4), np.float32); xywh_s[:n_real] = xywh[lo:]
            gx_s = np.zeros((R, 4), np.float32); gx_s[:n_real] = gx[lo:]
        pf = np.empty((128, PF_COLS), dtype=np.float32)
        pf[:, PF_LAB : PF_LAB + T] = lab_s.reshape(128, T)
        pf[:, PF_NLAB : PF_NLAB + T] = -lab_s.reshape(128, T)
        pf[:, PF_Z : PF_Z + T] = z_s.reshape(128, T)
        pf[:, PF_R : PF_R + T] = r_s.reshape(128, T)
        pf[:, PF_XYWH : PF_XYWH + 4 * T] = xywh_s.reshape(128, 4 * T)
        pf[:, PF_G : PF_G + 4 * T] = gx_s.reshape(128, 4 * T)
        pf[:, PF_IOTA : PF_IOTA + C] = iota_row
        in_maps.append({"cs": cs_s.reshape(128, T * C), "pf": pf})
    return in_maps


def combine_outputs(outs):
    """outs: list of [128, 2] per-core partials -> final [1] float32."""
    partA = float(sum(o[:, 0].astype(np.float64).sum() for o in outs))
    partB = float(sum(o[:, 1].astype(np.float64).sum() for o in outs))
    with np.errstate(over="ignore", under="ignore"):
        tps = np.exp(-partB)
    val = -partA + tps
    return np.array([val], dtype=np.float32)


_NC_CACHE = None


def get_nc():
    global _NC_CACHE
    if _NC_CACHE is None:
        _NC_CACHE = build_nc()
    return _NC_CACHE


def kernel(**inputs) -> np.ndarray:
    nc = get_nc()
    in_maps = make_in_maps(**inputs)
    res = run_bass_kernel_spmd(nc, in_maps, core_ids=list(range(NCORES)))
    return combine_outputs([res.results[c]["out"] for c in range(NCORES)])


# revision 13
# speedup vs baseline: 1.2500x; 1.0424x over previous
"""BPLoss Trainium2 kernel: 8-core SPMD over the detection (N) axis.

v6 design. Per core (shard of R=12544 rows; partition p owns rows
p*98..p*98+97):
  - tile groups (thirteen 7-tile groups + 3/2/2 tail), two plain HWDGE
    DMAs each: pool part [128,5,1024] on the ACT ring, DVE part
    [128,2,1024] on the SP ring (dual rings measured ~400 GB/s
    aggregate)
  - masking per 128-row tile:
      DVE part (2/group):  masked = (iota != label) * cs, fused stt to
                           scratch (1.28 us), per-part reduce_max
      pool part (5/group): PE-built corr block. Per tile: ACT builds the
          row one-hot O[p,m] = (m == nearest_gt_idx[row]) in two small
          [128,128] passes (Square -> Relu(1-u), bf16), PE transposes it
          and multiplies with the constant LT[m,c] = -BIG one-hot table,
          ACT copies the PSUM result into a bf16 corr block. One batched
          GpSimd add applies corr to cs in place (2.1 us/tile), then one
          contiguous grouped reduce_max. corr depends only on the index
          table, so it is built ahead of the data.
    The grouped reduce is deferred one group so the DVE's in-order queue
    never stalls on GpSimd.
  - epilogue: Ln on ScalarE, fused multiply-accumulate dot products for
    sum((z+r)*log_max) and sum(z*||xywh - gt_xywh[idx]||^2)
Host: shards, pads core 7, packs per-row tables and the LT/identity
constants, sums the 8x[128,2] partials, combines -A + exp(-B).
"""
import numpy as np
import ml_dtypes
import concourse.bass as bass
import concourse.tile as tile
from concourse import bacc, mybir
from concourse.bass_utils import run_bass_kernel_spmd

N, C, M = 100000, 1024, 128
NCORES = 8
T = 98              # 128-row tiles per core
R = T * 128         # 12544 rows per core
GROUPS = [(7, 2)] * 13 + [(3, 1), (2, 1), (2, 1)]
assert sum(g for g, _ in GROUPS) == T
POOL_BUFS = 4
DVE_BUFS = 3
NV_BUFS = 3
MSK_BUFS = 3
BIG = 1024.0
NBLK = C // 512     # PSUM banks per corr tile

f32 = mybir.dt.float32
bf16 = mybir.dt.bfloat16
OP = mybir.AluOpType
AF = mybir.ActivationFunctionType
AX = mybir.AxisListType

# packed f32 per-row tables: [label | -idx | z | r | xywh | g | iota]
PF_LAB = 0
PF_NIDX = T
PF_Z = 2 * T
PF_R = 3 * T
PF_XYWH = 4 * T
PF_G = 8 * T
PF_IOTA = 12 * T
PF_COLS = 12 * T + C
# packed bf16 constants: [LT | ident]
PB_LT = 0
PB_ID = C
PB_COLS = C + 128


def build_nc(reps=1):
    nc = bacc.Bacc("TRN2", target_bir_lowering=False, debug=False,
                   num_devices=NCORES)
    cs_d = nc.dram_tensor("cs", [128, T * C], f32, kind="ExternalInput").ap()
    pf_d = nc.dram_tensor("pf", [128, PF_COLS], f32, kind="ExternalInput").ap()
    pb_d = nc.dram_tensor("pb", [128, PB_COLS], bf16, kind="ExternalInput").ap()
    out_d = nc.dram_tensor("out", [128, 2], f32, kind="ExternalOutput").ap()

    with tile.TileContext(nc) as tc:
        with (
            tc.tile_pool(name="const", bufs=1) as constp,
            tc.tile_pool(name="dvep", bufs=DVE_BUFS) as dvep,
            tc.tile_pool(name="poolp", bufs=POOL_BUFS) as poolp,
            tc.tile_pool(name="nvp", bufs=NV_BUFS) as nvp,
            tc.tile_pool(name="op_", bufs=4) as op_,
            tc.tile_pool(name="psO", bufs=4, space="PSUM") as psO,
            tc.tile_pool(name="psC", bufs=2, space="PSUM") as psC,
            tc.tile_pool(name="mskp", bufs=MSK_BUFS) as mskp,
        ):
            pf = constp.tile([128, PF_COLS], f32)
            nc.scalar.dma_start(out=pf[:], in_=pf_d[:])
            pb = constp.tile([128, PB_COLS], bf16)
            nc.sync.dma_start(out=pb[:], in_=pb_d[:])
            LT = pb[:, PB_LT : PB_LT + C]
            ident = pb[:, PB_ID : PB_ID + 128]
            lab = pf[:, PF_LAB : PF_LAB + T]
            nidx = pf[:, PF_NIDX : PF_NIDX + T]
            z_sb = pf[:, PF_Z : PF_Z + T]
            r_sb = pf[:, PF_R : PF_R + T]
            xywh_sb = pf[:, PF_XYWH : PF_XYWH + 4 * T].rearrange(
                "p (t c) -> p t c", c=4
            )
            g_sb = pf[:, PF_G : PF_G + 4 * T].rearrange("p (t c) -> p t c", c=4)
            iota = pf[:, PF_IOTA : PF_IOTA + C]
            iota_m = pf[:, PF_IOTA : PF_IOTA + 128]   # 0..127 per partition

            w_sb = constp.tile([128, T], f32)
            nc.vector.tensor_add(w_sb[:], z_sb, r_sb)
            rowmax = constp.tile([128, T], f32)
            lm = constp.tile([128, T], f32)
            out_sb = constp.tile([128, 2], f32)
            scr = constp.tile([128, T], f32)
            scr2 = constp.tile([128, T], f32)
            diff = constp.tile([128, T, 4], f32)
            dsum = constp.tile([128, T], f32)

            def build_corr(t, dst):
                """dst [128, C] bf16 slice <- -BIG one-hot at label col."""
                u = op_.tile([128, 128], f32)
                nc.scalar.activation(
                    out=u[:], in_=iota_m, func=AF.Square,
                    scale=1.0, bias=nidx[:, t : t + 1],
                )
                O = op_.tile([128, 128], bf16)
                nc.scalar.activation(
                    out=O[:], in_=u[:], func=AF.Relu, scale=-1.0, bias=1.0,
                )
                OT_ps = psO.tile([128, 128], bf16)
                nc.tensor.transpose(OT_ps[:], O[:], ident)
                OT_sb = op_.tile([128, 128], bf16)
                nc.scalar.copy(out=OT_sb[:], in_=OT_ps[:])
                corr_ps = psC.tile([128, C], f32)
                for b in range(NBLK):
                    sl = slice(b * 512, (b + 1) * 512)
                    nc.tensor.matmul(
                        corr_ps[:, sl], OT_sb[:], LT[:, sl],
                        start=True, stop=True,
                    )
                nc.scalar.copy(out=dst, in_=corr_ps[:])

            for rep in range(reps):
                pend = None   # deferred grouped reduce (plw, td, npl)
                for gi, (gsz, nd) in enumerate(GROUPS):
                    t0 = sum(g for g, _ in GROUPS[:gi])
                    npl = gsz - nd
                    td = t0 + nd

                    # corr block for the pool part (pf/pb-only dep)
                    nv = nvp.tile([128, npl, C], bf16)
                    for k in range(npl):
                        build_corr(td + k, nv[:, k, :])

                    plw = poolp.tile([128, npl, C], f32)
                    nc.scalar.dma_start(
                        out=plw[:],
                        in_=cs_d[:, td * C : (td + npl) * C].rearrange(
                            "p (a c) -> p a c", c=C
                        ),
                    )
                    dvw = dvep.tile([128, nd, C], f32)
                    nc.sync.dma_start(
                        out=dvw[:],
                        in_=cs_d[:, t0 * C : td * C].rearrange(
                            "p (a c) -> p a c", c=C
                        ),
                    )

                    msk = mskp.tile([128, nd, C], f32)
                    for h in range(nd):
                        t = t0 + h
                        nc.vector.scalar_tensor_tensor(
                            out=msk[:, h, :], in0=iota,
                            scalar=lab[:, t : t + 1],
                            in1=dvw[:, h, :], op0=OP.not_equal, op1=OP.mult,
                        )
                    nc.vector.reduce_max(
                        rowmax[:, t0 : t0 + nd], msk[:], axis=AX.X
                    )
                    nc.gpsimd.tensor_tensor(
                        out=plw[:], in0=plw[:], in1=nv[:], op=OP.add,
                    )
                    if pend is not None:
                        pplw, ptd, pnpl = pend
                        nc.vector.reduce_max(
                            rowmax[:, ptd : ptd + pnpl], pplw[:], axis=AX.X
                        )
                    pend = (plw, td, npl)
                pplw, ptd, pnpl = pend
                nc.vector.reduce_max(
                    rowmax[:, ptd : ptd + pnpl], pplw[:], axis=AX.X
                )

                # epilogue: partial sums
                nc.scalar.activation(out=lm[:], in_=rowmax[:], func=AF.Ln)
                nc.vector.scalar_tensor_tensor(
                    out=scr[:], in0=w_sb[:], scalar=0.0, in1=lm[:],
                    op0=OP.bypass, op1=OP.mult, accum_out=out_sb[:, 0:1],
                )
                nc.vector.tensor_sub(diff[:], xywh_sb, g_sb)
                nc.vector.tensor_mul(diff[:], diff[:], diff[:])
                nc.vector.reduce_sum(dsum[:], diff[:], axis=AX.X)
                nc.vector.scalar_tensor_tensor(
                    out=scr2[:], in0=z_sb, scalar=0.0, in1=dsum[:],
                    op0=OP.bypass, op1=OP.mult, accum_out=out_sb[:, 1:2],
                )
            nc.sync.dma_start(out=out_d[:], in_=out_sb[:])

    nc.compile()
    return nc


def make_in_maps(class_scores, xywh, z, r, nearest_gt_idx, gt_class_labels, gt_xywh):
    cs = np.ascontiguousarray(np.asarray(class_scores, dtype=np.float32))
    xywh = np.ascontiguousarray(np.asarray(xywh, dtype=np.float32))
    z = np.ascontiguousarray(np.asarray(z, dtype=np.float32))
    r = np.ascontiguousarray(np.asarray(r, dtype=np.float32))
    idx = np.asarray(nearest_gt_idx).astype(np.int64)
    gtl = np.asarray(gt_class_labels).astype(np.int64)
    labels = gtl.astype(np.float32)[idx]                               # [N]
    gx = np.asarray(gt_xywh, dtype=np.float32)[idx]                    # [N,4]

    pb = np.zeros((128, PB_COLS), dtype=ml_dtypes.bfloat16)
    pb[:, PB_LT : PB_LT + C] = (
        -BIG * (gtl[:, None] == np.arange(C)[None, :])
    ).astype(ml_dtypes.bfloat16)
    pb[:, PB_ID : PB_ID + 128] = np.eye(128, dtype=ml_dtypes.bfloat16)

    iota_row = np.arange(C, dtype=np.float32)[None, :]
    in_maps = []
    for c in range(NCORES):
        lo, hi = c * R, (c + 1) * R
        if hi <= N:
            cs_s = cs[lo:hi]
            lab_s, z_s, r_s = labels[lo:hi], z[lo:hi], r[lo:hi]
            idx_s = idx[lo:hi]
            xywh_s, gx_s = xywh[lo:hi], gx[lo:hi]
        else:
            n_real = N - lo
            cs_s = np.ones((R, C), dtype=np.float32)
            cs_s[:n_real] = cs[lo:]
            lab_s = np.zeros(R, np.float32); lab_s[:n_real] = labels[lo:]
            idx_s = np.zeros(R, np.int64); idx_s[:n_real] = idx[lo:]
            z_s = np.zeros(R, np.float32); z_s[:n_real] = z[lo:]
            r_s = np.zeros(R, np.float32); r_s[:n_real] = r[lo:]
            xywh_s = np.zeros((R, 4), np.float32); xywh_s[:n_real] = xywh[lo:]
            gx_s = np.zeros((R, 4), np.float32); gx_s[:n_real] = gx[lo:]
        pf = np.empty((128, PF_COLS), dtype=np.float32)
        pf[:, PF_LAB : PF_LAB + T] = lab_s.reshape(128, T)
        pf[:, PF_NIDX : PF_NIDX + T] = -idx_s.reshape(128, T).astype(np.float32)
        pf[:, PF_Z : PF_Z + T] = z_s.reshape(128, T)
        pf[:, PF_R : PF_R + T] = r_s.reshape(128, T)
        pf[:, PF_XYWH : PF_XYWH + 4 * T] = xywh_s.reshape(128, 4 * T)
        pf[:, PF_G : PF_G + 4 * T] = gx_s.reshape(128, 4 * T)
        pf[:, PF_IOTA : PF_IOTA + C] = iota_row
        in_maps.append({"cs": cs_s.reshape(128, T * C), "pf": pf, "pb": pb})
    return in_maps


def combine_outputs(outs):
    """outs: list of [128, 2] per-core partials -> final [1] float32."""
    partA = float(sum(o[:, 0].astype(np.float64).sum() for o in outs))
    partB = float(sum(o[:, 1].astype(np.float64).sum() for o in outs))
    with np.errstate(over="ignore", under="ignore"):
        tps = np.exp(-partB)
    val = -partA + tps
    return np.array([val], dtype=np.float32)


_NC_CACHE = None


def get_nc():
    global _NC_CACHE
    if _NC_CACHE is None:
        _NC_CACHE = build_nc()
    return _NC_CACHE


def kernel(**inputs) -> np.ndarray:
    nc = get_nc()
    in_maps = make_in_maps(**inputs)
    res = run_bass_kernel_spmd(nc, in_maps, core_ids=list(range(NCORES)))
    return combine_outputs([res.results[c]["out"] for c in range(NCORES)])
